# revision 12
# baseline (speedup 1.0000x reference)
"""Trainium2 Bass kernel for nn_CrossAttention2D.

Reference computation (per batch b, row h):
    Q = w1 @ Xw + b1          (Xw = waveform[b,:,h,:]  [C=128, W=512])
    K = w2 @ Xs + b2          (Xs = spectrogram[b,:,h,:])
    S = Q^T K * 1/sqrt(F)     [512, 512]
    P = softmax(S, axis=-1)
    out[b,:,h,:] = Xs @ P^T   [C, W]

Device algorithm (S^T layout so the softmax bias folds into the exp):
    S^T = Xs^T (M^T Xw) + v 1^T  (+ q-constant terms that cancel in softmax)
with M = w1^T w2, v = Xs^T (w2^T b1).  Per 128-row k-chunk:
    T2 = M^T Xw                       (1 matmul, lhsT = M)
    st_kc = Xs[:,kc]^T T2             (lhsT = Xs chunk, no transposes)
    v_kc  = Xs[:,kc]^T beta           (same stationary weights)
    p_kc  = exp(scale*st_kc + scale*v_kc)       (ACT, bias per-partition)
    rb   += ones128 @ p_kc            (row sums, pre-broadcast to 128 rows)
    o    += XsT[kc] @ p_kc            (XsT pre-transposed on host)
    out   = o * reciprocal(rb)
All matmul operands are bf16 (host-converted, so no on-device dtype
legalization copies); accumulation is fp32 in PSUM.  Inputs are DMA'd in
8-row blocks, outputs stored in 4-row blocks.

The emission is software-pipelined one h ahead: body(i) issues the
denominator/AV matmuls of row i interleaved with the score matmuls + exp
of row i+1 and the T2 matmul of row i+2, so every PE instruction's
dependencies are one body old and the PE streams without stalls.

Sharding: data-parallel over batch B=8 across 8 NeuronCores (one batch
image per core, small weights replicated). No collectives.
"""

import contextlib

import ml_dtypes
import numpy as np

import concourse.bacc as bacc
import concourse.tile as tile
from concourse import mybir
from concourse.bass_utils import run_bass_kernel_spmd

B = 8
C = 128  # channel dim (TIME_DIM == SPEC_DIM == 128)
H = 64
W = 512
N_CORES = 8
SCALE = 1.0 / 16.0  # 1/sqrt(FEATURE_DIM=256)
HB = 8  # h rows per input DMA block
OB = 4  # h rows per output DMA block

FP32 = mybir.dt.float32
BF16 = mybir.dt.bfloat16
EXP = mybir.ActivationFunctionType.Exp
MULT = mybir.AluOpType.mult


def build_module(n_h=H, rep=1):
    """Build the per-core Bass module processing [C, n_h, W] inputs.

    rep > 1 repeats the computation on the same data (timing runs only).
    """
    assert n_h % HB == 0 and HB % OB == 0
    nc = bacc.Bacc("TRN2", target_bir_lowering=False, debug=False)

    wave = nc.dram_tensor("wave", [C, n_h, W], BF16, kind="ExternalInput").ap()
    spec = nc.dram_tensor("spec", [C, n_h, W], BF16, kind="ExternalInput").ap()
    # V^T blocks: st4[w0, h, j, c] = spec[c, h, 128*j + w0]
    st4 = nc.dram_tensor("st4", [C, n_h, 4, C], BF16, kind="ExternalInput").ap()
    # mt = w1^T w2 so that matmul's lhsT.T = M^T
    mt = nc.dram_tensor("mt", [C, C], BF16, kind="ExternalInput").ap()
    # beta = w2^T b1 (the only bias term that survives softmax invariance)
    beta = nc.dram_tensor("beta", [C, 1], BF16, kind="ExternalInput").ap()
    out = nc.dram_tensor("out", [C, n_h, W], FP32, kind="ExternalOutput").ap()

    n_blk = n_h // HB

    with tile.TileContext(nc) as tc:
        with (
            tc.tile_pool(name="consts", bufs=1) as consts,
            tc.tile_pool(name="io", bufs=3) as io,
            tc.tile_pool(name="ob", bufs=2) as ob,
            tc.tile_pool(name="work", bufs=3) as work,
            tc.tile_pool(name="small", bufs=4) as small,
            tc.tile_pool(name="pt2", bufs=1, space="PSUM") as pt2,
            tc.tile_pool(name="pst", bufs=2, space="PSUM") as pst,
            tc.tile_pool(name="prb", bufs=2, space="PSUM") as prb,
            tc.tile_pool(name="po", bufs=2, space="PSUM") as po,
            tc.tile_pool(name="pv", bufs=1, space="PSUM") as pv,
        ):
            mt_sb = consts.tile([C, C], BF16, tag="mt")
            nc.sync.dma_start(mt_sb, mt)
            beta_sb = consts.tile([C, 1], BF16, tag="beta")
            nc.sync.dma_start(beta_sb, beta)
            ones_sb = consts.tile([C, C], BF16, tag="ones")
            nc.vector.memset(ones_sb, 1.0)

            rep_ctx = tc.For_i(0, rep, 1) if rep > 1 else contextlib.nullcontext()
            with rep_ctx:
                blocks = {}  # blk index -> (wave, spec, st4) tiles

                def load_blk(blk):
                    if blk in blocks or blk >= n_blk:
                        return
                    h0 = blk * HB
                    wv_t = io.tile([C, HB, W], BF16, tag="wv")
                    nc.sync.dma_start(wv_t, wave[:, h0 : h0 + HB, :])
                    sp_t = io.tile([C, HB, W], BF16, tag="sp")
                    nc.sync.dma_start(sp_t, spec[:, h0 : h0 + HB, :])
                    s4_t = io.tile([C, HB, 4, C], BF16, tag="s4")
                    nc.sync.dma_start(s4_t, st4[:, h0 : h0 + HB, :, :])
                    blocks[blk] = (wv_t, sp_t, s4_t)
                    if blk - 3 in blocks:
                        del blocks[blk - 3]

                def t2_mm(h):
                    """T2(h) = M^T Xw: matmul into PSUM."""
                    wv = blocks[h // HB][0][:, h % HB, :]
                    t2_ps = pt2.tile([C, W], FP32, tag="t2")
                    nc.tensor.matmul(t2_ps, mt_sb, wv, start=True, stop=True)
                    return t2_ps

                def t2_copy(t2_ps):
                    t2_sb = work.tile([C, W], BF16, tag="t2")
                    nc.vector.tensor_copy(t2_sb, t2_ps)
                    return t2_sb

                def t2_start(h):
                    return t2_copy(t2_mm(h))

                def st_chunk(h, kc, t2_sb, expst, vb, vcol_ps):
                    """Score matmul + bias matmul + exp for chunk kc of row h."""
                    sp = blocks[h // HB][1][:, h % HB, :]
                    kblk = slice(kc * 128, (kc + 1) * 128)
                    st_ps = pst.tile([C, W], FP32, tag="st")
                    nc.tensor.matmul(st_ps, sp[:, kblk], t2_sb, start=True, stop=True)
                    nc.tensor.matmul(
                        vcol_ps[:, kc : kc + 1],
                        sp[:, kblk],
                        beta_sb,
                        start=True,
                        stop=True,
                    )
                    nc.vector.tensor_scalar_mul(
                        vb[:, kc : kc + 1], vcol_ps[:, kc : kc + 1], SCALE
                    )
                    nc.scalar.activation(
                        expst[:, kc, :],
                        st_ps,
                        EXP,
                        bias=vb[:, kc : kc + 1],
                        scale=SCALE,
                    )

                def st_phase_alloc():
                    expst = work.tile([C, 4, W], BF16, tag="p")
                    vb = small.tile([C, 4], FP32, tag="vb")
                    vcol_ps = pv.tile([C, 4], FP32, tag="v")
                    return expst, vb, vcol_ps

                # ---- prologue: rows 0 and 1 score phases ----
                load_blk(0)
                load_blk(1)
                t2_cur = t2_start(0)  # t2 for row 0
                cur_exp = st_phase_alloc()
                for kc in range(4):
                    st_chunk(0, kc, t2_cur, *cur_exp)
                t2_cur = t2_start(1) if n_h > 1 else None

                o_blks = {}
                finish_prev = None  # closure finalizing row i-1 (recip+tt+dma)
                # ---- steady bodies ----
                for i in range(n_h):
                    if i % HB == 0:
                        load_blk(i // HB + 2)
                    if i % OB == 0:
                        o_blks[i // OB] = ob.tile(
                            [C, OB, W], FP32, tag="o", name="o_blk"
                        )
                        if i // OB - 2 in o_blks:
                            del o_blks[i // OB - 2]
                    expst_i = cur_exp[0]
                    nxt_exp = st_phase_alloc() if i + 1 < n_h else None
                    rb_ps = prb.tile([C, W], FP32, tag="rb")
                    o_ps = po.tile([C, W], FP32, tag="o")
                    st4_i = blocks[i // HB][2]
                    for kc in range(4):
                        nc.tensor.matmul(
                            rb_ps,
                            ones_sb,
                            expst_i[:, kc, :],
                            start=(kc == 0),
                            stop=(kc == 3),
                        )
                        nc.tensor.matmul(
                            o_ps,
                            st4_i[:, i % HB, kc, :],
                            expst_i[:, kc, :],
                            start=(kc == 0),
                            stop=(kc == 3),
                        )
                        if nxt_exp is not None:
                            if kc == 3 and i + 2 < n_h:
                                t2_ps_nxt = t2_mm(i + 2)
                            st_chunk(i + 1, kc, t2_cur, *nxt_exp)
                        if kc == 0 and finish_prev is not None:
                            # row i-1 epilogue lands after vb0(i+1) on DVE so
                            # the exp chain is never queued behind it
                            finish_prev()
                            finish_prev = None
                    if nxt_exp is not None and i + 2 < n_h:
                        t2_cur = t2_copy(t2_ps_nxt)
                    cur_exp = nxt_exp

                    def finish_row(i=i, rb_ps=rb_ps, o_ps=o_ps):
                        rcb_sb = work.tile([C, W], FP32, tag="rcb")
                        nc.vector.reciprocal(rcb_sb, rb_ps)
                        o_blk = o_blks[i // OB]
                        nc.vector.tensor_tensor(
                            o_blk[:, i % OB, :], o_ps, rcb_sb, op=MULT
                        )
                        if i % OB == OB - 1:
                            h1 = i - (OB - 1)
                            nc.sync.dma_start(out[:, h1 : h1 + OB, :], o_blk)

                    finish_prev = finish_row
                if finish_prev is not None:
                    finish_prev()

    nc.compile()
    return nc


def host_prep(waveform, spectrogram, w1, b1, w2, b2):
    """Precompute host-side tensors (float64 for the small weight algebra)."""
    w1d = np.asarray(w1, np.float64)
    w2d = np.asarray(w2, np.float64)
    b1d = np.asarray(b1, np.float64)
    mt = np.ascontiguousarray((w1d.T @ w2d).astype(ml_dtypes.bfloat16))
    beta = np.ascontiguousarray((w2d.T @ b1d)[:, None].astype(ml_dtypes.bfloat16))
    wave_bf = np.asarray(waveform, ml_dtypes.bfloat16)
    spec_bf = np.asarray(spectrogram, ml_dtypes.bfloat16)
    # st4[b, w0, h, j, c] = spec[b, c, h, 128*j + w0]
    st4 = np.ascontiguousarray(
        spec_bf.reshape(B, C, H, 4, 128).transpose(0, 4, 2, 3, 1)
    )
    return wave_bf, spec_bf, st4, mt, beta


_NC_CACHE = {}


def _get_nc(n_h=H, rep=1):
    key = (n_h, rep)
    if key not in _NC_CACHE:
        _NC_CACHE[key] = build_module(n_h, rep)
    return _NC_CACHE[key]


def run_device(waveform, spectrogram, w1, b1, w2, b2, n_h=H, rep=1, **run_kwargs):
    """Shard over batch, run on 8 cores, gather. Returns (output, results)."""
    wave_bf, spec_bf, st4, mt, beta = host_prep(
        waveform, spectrogram, w1, b1, w2, b2
    )

    in_maps = [
        {
            "wave": np.ascontiguousarray(wave_bf[b, :, :n_h, :]),
            "spec": np.ascontiguousarray(spec_bf[b, :, :n_h, :]),
            "st4": np.ascontiguousarray(st4[b, :, :n_h, :, :]),
            "mt": mt,
            "beta": beta,
        }
        for b in range(B)
    ]
    nc = _get_nc(n_h, rep)
    res = run_bass_kernel_spmd(nc, in_maps, core_ids=list(range(N_CORES)), **run_kwargs)
    output = np.stack([res.results[b]["out"] for b in range(B)], axis=0)
    return output, res


def kernel(waveform, spectrogram, w1, b1, w2, b2):
    output, _ = run_device(waveform, spectrogram, w1, b1, w2, b2)
    return output.astype(np.float32)
